# revision 5
# baseline (speedup 1.0000x reference)
"""MoE-routed per-sample conv2d kernel for Trainium2 (8 NeuronCores, SPMD).

Math (per sample b):
    y_ctx  = mean(y[b], HW)                              [C]
    gates  = softmax(y_ctx @ (gate_w[:C] + gate_w[C:]) + gate_b)   [E]
    Wf[e]  = experts[e,:, :C] + experts[e,:, C:]         [O, C, K, K]  (fold of q;q concat)
    agg    = sum_e gates[e] * Wf[e]
    out[b] = conv2d(q[b], agg, SAME)

Sharding: data-parallel over batch. Each of the 8 cores handles B/8 = 2
samples; experts/gate params replicated. Conv runs on the TensorEngine as
9 shifted matmuls (one per kernel tap) accumulated in PSUM, fp32r.

Engine roles:
  SP (sync)      bulk-load DMA ring A, FIFO-chained
  Pool (gpsimd)  bulk-load DMA ring B (SWDGE) + xc guard memsets + tiny loads
  ACT (scalar)   startup: 4 y0 chunks on its (otherwise idle) HWDGE ring +
                 wft drains + exp; steady: PSUM->SBUF output copies and the
                 output-write triggers (ring carries no steady-state loads)
  DVE (vector)   y reduces, gating vector ops, agg combine (FMA), err subs
  PE (tensor)    folded expert transposes (accumulating), gating matmuls, conv
"""

import numpy as np

import concourse.bass as bass
import concourse.tile as tile
from concourse import bacc, mybir
from concourse.bass_utils import run_bass_kernel_spmd
from concourse.masks import make_identity
from concourse.tile_rust import add_dep_helper

F32 = mybir.dt.float32
F32R = mybir.dt.float32r

B, C, O, H, W, E, K = 16, 128, 128, 128, 128, 3, 3
NCORES = 8
BPC = B // NCORES          # samples per core
CH_ROWS = 32               # output rows per conv chunk
NCH = H // CH_ROWS         # chunks per sample (4)
RB_ROWS = 4                # output rows per PSUM block (4*128 = 512 free)
NRB = CH_ROWS // RB_ROWS   # row blocks per chunk (8)
HCH = CH_ROWS // 2         # rows per output write (half chunk)
XCF = 2 + (CH_ROWS + 3) * W      # flat chunk tile: 2 lead zeros, 35 rows, slack
YCHUNK = 2048              # y columns per reduce chunk (1 MB)
NYCH = (H * W) // YCHUNK   # 8

# taps ordered so the first one covers the full output range (ky=1,kx=1)
TAPS = [(1, 1)] + [(ky, kx) for ky in range(3) for kx in range(3) if (ky, kx) != (1, 1)]

MUL = mybir.AluOpType.mult
ADD = mybir.AluOpType.add


def build_nc():
    nc = bacc.Bacc(None, target_bir_lowering=False)

    q_d = nc.dram_tensor("q", [BPC, C, H, W], F32, kind="ExternalInput")
    y_d = nc.dram_tensor("y", [BPC, C, H, W], F32, kind="ExternalInput")
    ex_d = nc.dram_tensor("experts", [E, O, 2 * C, K, K], F32, kind="ExternalInput")
    gw_d = nc.dram_tensor("gate_w", [2 * C, E], F32, kind="ExternalInput")
    gb_d = nc.dram_tensor("gate_b", [E], F32, kind="ExternalInput")
    out_d = nc.dram_tensor("out", [BPC, O, H, W], F32, kind="ExternalOutput")

    with tile.TileContext(nc) as tc:
        import contextlib

        with contextlib.ExitStack() as ctx:
            const = ctx.enter_context(tc.tile_pool(name="const", bufs=1))
            wraw = ctx.enter_context(tc.tile_pool(name="wraw", bufs=3))
            wft = ctx.enter_context(tc.tile_pool(name="wft", bufs=3))
            ypool = ctx.enter_context(tc.tile_pool(name="ypool", bufs=6))
            gp = ctx.enter_context(tc.tile_pool(name="gp", bufs=4))
            atmp = ctx.enter_context(tc.tile_pool(name="atmp", bufs=1))
            aggp = ctx.enter_context(tc.tile_pool(name="aggp", bufs=2))
            xcp = ctx.enter_context(tc.tile_pool(name="xcp", bufs=4))
            osbp = ctx.enter_context(tc.tile_pool(name="osbp", bufs=3))
            psp = ctx.enter_context(tc.tile_pool(name="psp", bufs=6, space="PSUM"))
            pse = ctx.enter_context(tc.tile_pool(name="pse", bufs=2, space="PSUM"))

            # Keep each DMA ring's transfer order exactly as emitted: the
            # static Tile scheduler otherwise floats "ready" q-chunk loads
            # ahead of y chunks whose DMA waits on a pool slot, starving the
            # gating path.
            last_dma = {}

            def chained_dma(eng, out, in_):
                inst = eng.dma_start(out=out, in_=in_)
                key = eng.engine
                if key in last_dma:
                    add_dep_helper(inst.ins, last_dma[key], sync=False,
                                   reason="ring FIFO order")
                last_dma[key] = inst.ins
                return inst

            # ---- tiny loads + constants ------------------------------------
            gw = const.tile([C, 2, E], F32, tag="gw", name="gw")
            chained_dma(nc.gpsimd, gw[:], gw_d[:].rearrange("(h c) e -> c h e", h=2))
            gbt = const.tile([1, E], F32, tag="gbt", name="gbt")
            chained_dma(nc.gpsimd, gbt[:], gb_d[:].rearrange("(x e) -> x e", x=1))

            # expert loads: e0 on SP; e1, e2 on gpsimd ring
            wes = []
            for e, eng in ((0, nc.sync), (1, nc.gpsimd), (2, nc.gpsimd)):
                we = wraw.tile([O, 2 * C, K, K], F32, tag="wraw", name=f"we{e}")
                chained_dma(eng, we[:], ex_d[e])
                wes.append(we)

            # ---- y0 loads (emitted early so every ring's triggers sit at
            # the head of its FIFO; ACT's otherwise-idle ring helps out) -----
            yflat = y_d[:].rearrange("b c h w -> b c (h w)")
            HALF = YCHUNK // 2
            y0_cs = []
            for j in range(3):
                yc = ypool.tile([C, YCHUNK], F32, tag="yc", name=f"yc0_{j}")
                chained_dma(nc.sync, yc[:], yflat[0, :, j * YCHUNK:(j + 1) * YCHUNK])
                y0_cs.append(yc)
            for j in range(3, 7):
                yc = ypool.tile([C, YCHUNK], F32, tag="yc", name=f"yc0_{j}")
                chained_dma(nc.scalar, yc[:], yflat[0, :, j * YCHUNK:(j + 1) * YCHUNK])
                y0_cs.append(yc)
            yc7 = ypool.tile([C, YCHUNK], F32, tag="yc", name="yc0_7")
            for h in range(2):
                chained_dma(nc.gpsimd, yc7[:, h * HALF:(h + 1) * HALF],
                            yflat[0, :, 7 * YCHUNK + h * HALF:7 * YCHUNK + (h + 1) * HALF])
            y0_cs.append(yc7)

            ident = const.tile([128, 128], F32, tag="ident", name="ident")
            make_identity(nc, ident)

            # prewarm the ACT Exp table so gating doesn't pay the table load
            warm = const.tile([1, 1], F32, tag="warm", name="warm")
            nc.vector.memset(warm[:], 0.0)
            nc.scalar.activation(warm[:], warm[:], mybir.ActivationFunctionType.Exp,
                                 bias=0.0, scale=1.0)

            ones = const.tile([1, 128], F32, tag="ones", name="ones")
            nc.vector.memset(ones[:], 1.0)

            weff = const.tile([C, E], F32, tag="weff", name="weff")
            nc.vector.tensor_add(weff[:], gw[:, 0, :], gw[:, 1, :])
            # fold the 1/HW of the y-mean into the gate weight
            nc.vector.tensor_scalar_mul(weff[:], weff[:], 1.0 / float(H * W))

            # ---- expert transpose with in-PE fold --------------------------
            # agg lhsT layout [c, t, o]; fold of the duplicated input halves
            # done by two accumulating PE transposes per tap (no DVE fold).
            wfts = []
            for e in range(E):
                we = wes[e]
                wt = wft.tile([C, K * K, O], F32, tag="wft", name=f"wft{e}")
                for t, (ky, kx) in enumerate(TAPS):
                    pst = psp.tile([128, 128], F32, tag="ps", name=f"pst{e}_{t}")
                    nc.tensor.matmul(pst[:], we[:, 0:C, ky, kx], ident[:],
                                     is_transpose=True, start=True, stop=False)
                    nc.tensor.matmul(pst[:], we[:, C:2 * C, ky, kx], ident[:],
                                     is_transpose=True, start=False, stop=True)
                    nc.scalar.copy(wt[:, t, :], pst[:])
                wfts.append(wt)

            # ---- y reduction (all on DVE) ----------------------------------
            ysums = []
            yparts = []

            def emit_y_reduces(b, ycs, js):
                while b >= len(yparts):
                    ypart = gp.tile([C, NYCH + 1], F32, tag="ypart",
                                    name=f"ypart{len(yparts)}")
                    yparts.append(ypart)
                for j in js:
                    if j == 7 and b == 0:
                        # split tail chunk: two short reduces cut the tail
                        nc.vector.reduce_sum(yparts[b][:, 7:8], ycs[7][:, 0:HALF],
                                             axis=mybir.AxisListType.X)
                        nc.vector.reduce_sum(yparts[b][:, 8:9], ycs[7][:, HALF:],
                                             axis=mybir.AxisListType.X)
                    else:
                        nc.vector.reduce_sum(yparts[b][:, j:j + 1], ycs[j][:],
                                             axis=mybir.AxisListType.X)

            # ---- q chunk staging -------------------------------------------
            xcs = {}

            def load_xc(b, ch, eng):
                xr_lo = max(0, CH_ROWS * ch - 1)
                xr_hi = min(H - 1, CH_ROWS * ch + CH_ROWS)
                nrows = xr_hi - xr_lo + 1
                j0 = xr_lo - (CH_ROWS * ch - 1)
                xc = xcp.tile([C, XCF], F32R, tag="xc", name=f"xc{b}_{ch}")
                nc.gpsimd.memset(xc[:, 0:2].bitcast(F32), 0.0)
                nc.gpsimd.memset(
                    xc[:, 2 + (CH_ROWS + 2) * W: 2 + (CH_ROWS + 2) * W + 2].bitcast(F32), 0.0)
                if ch == 0:
                    nc.gpsimd.memset(xc[:, 2:2 + W].bitcast(F32), 0.0)
                if ch == NCH - 1:
                    nc.gpsimd.memset(
                        xc[:, 2 + (CH_ROWS + 1) * W: 2 + (CH_ROWS + 2) * W].bitcast(F32), 0.0)
                chained_dma(
                    eng,
                    xc[:, 2 + j0 * W: 2 + (j0 + nrows) * W],
                    q_d[b, :, xr_lo:xr_hi + 1, :].rearrange("c h w -> c (h w)").bitcast(F32R),
                )
                xcs[(b, ch)] = xc

            # ---- gating + weight aggregation per sample --------------------
            aggs = []

            def gate_and_agg(b):
                ncols = NYCH + 1 if b == 0 else NYCH
                ysum = gp.tile([C, 1], F32, tag="ysum", name=f"ysum{b}")
                nc.vector.reduce_sum(ysum[:], yparts[b][:, 0:ncols],
                                     axis=mybir.AxisListType.X)
                ysums.append(ysum)
                ps13 = pse.tile([1, E], F32, tag="pse", name=f"ps13_{b}")
                nc.tensor.matmul(ps13[:], ysums[b][:], weff[:], start=True, stop=True)
                logits = gp.tile([1, E], F32, tag="logits", name=f"logits{b}")
                nc.vector.tensor_add(logits[:], ps13[:], gbt[:])
                mx = gp.tile([1, 1], F32, tag="mx", name=f"mx{b}")
                nc.vector.reduce_max(mx[:], logits[:], axis=mybir.AxisListType.X)
                nc.vector.tensor_scalar_mul(mx[:], mx[:], -1.0)
                nc.scalar.activation(logits[:], logits[:], mybir.ActivationFunctionType.Exp,
                                     bias=mx[:], scale=1.0)
                sm = gp.tile([1, 1], F32, tag="sm", name=f"sm{b}")
                nc.vector.reduce_sum(sm[:], logits[:], axis=mybir.AxisListType.X)
                nc.vector.reciprocal(sm[:], sm[:])
                nc.vector.tensor_scalar_mul(logits[:], logits[:], sm[:])
                # broadcast gates to all partitions via a K=1 matmul with ones
                psg = pse.tile([128, E], F32, tag="pse", name=f"psg{b}")
                nc.tensor.matmul(psg[:], ones[:], logits[:], start=True, stop=True)
                gbc = gp.tile([128, E], F32, tag="gbc", name=f"gbc{b}")
                nc.vector.tensor_copy(gbc[:], psg[:])

                # aggregate in 3 tap-groups so the first conv matmuls can
                # start while later groups still combine; a mul plus two DVE
                # FMAs (scalar_tensor_tensor) per group
                accf = atmp.tile([C, K * K, O], F32, tag="accf", name=f"accf{b}")
                agg = aggp.tile([C, K * K, O], F32R, tag="agg", name=f"agg{b}")
                for g3 in range(3):
                    sl = slice(3 * g3, 3 * g3 + 3)
                    nc.vector.tensor_scalar_mul(accf[:, sl, :], wfts[0][:, sl, :],
                                                gbc[:, 0:1])
                    nc.vector.scalar_tensor_tensor(
                        accf[:, sl, :], wfts[1][:, sl, :], gbc[:, 1:2],
                        accf[:, sl, :], MUL, ADD)
                    nc.vector.scalar_tensor_tensor(
                        agg[:, sl, :], wfts[2][:, sl, :], gbc[:, 2:3],
                        accf[:, sl, :], MUL, ADD)
                aggs.append(agg)

            # ---- conv ------------------------------------------------------
            # Main taps read the flat chunk at offset 2 + (4rb+ky)*W + kx-1.
            # For kx=0 the first column of each row wrongly reads the last
            # element of the previous row (and vice versa for kx=2), which
            # SAME-padding says should be zero.  err matmuls compute exactly
            # those wrong contributions; they are subtracted on the SBUF copy.
            def conv_chunk(b, ch):
                last = (b == BPC - 1) and (ch == NCH - 1)
                xc = xcs[(b, ch)]
                x1 = xc[:, 1:1 + (CH_ROWS + 2) * W].rearrange("c (r w) -> c r w", w=W)
                x2 = xc[:, 2:2 + (CH_ROWS + 3) * W].rearrange("c (r w) -> c r w", w=W)
                # err psum [O, 2, CH_ROWS]: group 0 = col 0, group 1 = col W-1
                errps = pse.tile([O, 2, CH_ROWS], F32, tag="pse", name=f"eps{b}_{ch}")
                first = True
                for t, (ky, kx) in enumerate(TAPS):
                    if kx == 1:
                        continue
                    if kx == 0:
                        g, rhs = 0, x1[:, ky:ky + CH_ROWS, 0:1]
                    else:
                        g, rhs = 1, x2[:, ky + 1:ky + 1 + CH_ROWS, 0:1]
                    nc.tensor.matmul(
                        errps[:, g, :], aggs[b][:, t, :], rhs,
                        start=first, stop=(t == len(TAPS) - 1), skip_group_check=True,
                    )
                    first = False
                # two half-chunk output stages, each its own SBUF tile + write
                for hh in range(2):
                    osb = osbp.tile([O, HCH, W], F32, tag="osb",
                                    name=f"osb{b}_{ch}_{hh}")
                    for rb in range(hh * NRB // 2, (hh + 1) * NRB // 2):
                        ps = psp.tile([O, RB_ROWS, W], F32, tag="ps",
                                      name=f"ps{b}_{ch}_{rb}")
                        for t, (ky, kx) in enumerate(TAPS):
                            jb = RB_ROWS * rb + ky
                            off = 2 + jb * W + kx - 1
                            rhs = xc[:, off:off + RB_ROWS * W]  # contiguous 512
                            nc.tensor.matmul(
                                ps[:],
                                aggs[b][:, t, :],
                                rhs,
                                start=(t == 0),
                                stop=(t == len(TAPS) - 1),
                            )
                        osl = slice(RB_ROWS * rb - hh * HCH,
                                    RB_ROWS * (rb + 1) - hh * HCH)
                        esl = slice(RB_ROWS * rb, RB_ROWS * (rb + 1))
                        nc.scalar.copy(osb[:, osl, :], ps[:])
                        if last and hh == 1:
                            # drain the tail per row-block to cut the drain
                            nc.vector.tensor_sub(osb[:, osl, 0], osb[:, osl, 0],
                                                 errps[:, 0, esl])
                            nc.vector.tensor_sub(osb[:, osl, W - 1],
                                                 osb[:, osl, W - 1],
                                                 errps[:, 1, esl])
                            r0 = CH_ROWS * ch + RB_ROWS * rb
                            chained_dma(nc.scalar,
                                        out_d[b, :, r0:r0 + RB_ROWS, :],
                                        osb[:, osl, :])
                    if not (last and hh == 1):
                        esl = slice(hh * HCH, (hh + 1) * HCH)
                        nc.vector.tensor_sub(osb[:, :, 0], osb[:, :, 0],
                                             errps[:, 0, esl])
                        nc.vector.tensor_sub(osb[:, :, W - 1], osb[:, :, W - 1],
                                             errps[:, 1, esl])
                        r0 = CH_ROWS * ch + hh * HCH
                        chained_dma(nc.scalar, out_d[b, :, r0:r0 + HCH, :], osb[:])

            # ---- schedule --------------------------------------------------
            load_xc(0, 0, nc.sync)
            load_xc(0, 1, nc.gpsimd)
            emit_y_reduces(0, y0_cs, range(NYCH))
            gate_and_agg(0)
            conv_chunk(0, 0)
            # y1 rides SP/Q0 behind the first q chunks; ACT stays writes-only
            y1_cs = []
            for j in range(NYCH):
                yc = ypool.tile([C, YCHUNK], F32, tag="yc", name=f"yc1_{j}")
                chained_dma(nc.sync if j < 4 else nc.gpsimd, yc[:],
                            yflat[1, :, j * YCHUNK:(j + 1) * YCHUNK])
                y1_cs.append(yc)
            load_xc(0, 2, nc.sync)
            load_xc(0, 3, nc.gpsimd)
            conv_chunk(0, 1)
            emit_y_reduces(1, y1_cs, range(0, 5))
            load_xc(1, 0, nc.sync)
            load_xc(1, 1, nc.gpsimd)
            conv_chunk(0, 2)
            emit_y_reduces(1, y1_cs, range(5, 8))
            gate_and_agg(1)
            load_xc(1, 2, nc.sync)
            load_xc(1, 3, nc.gpsimd)
            conv_chunk(0, 3)
            for ch in range(NCH):
                conv_chunk(1, ch)

    nc.compile()
    return nc


_NC_CACHE = None


def kernel(q, y, experts, gate_w, gate_b, _trace=False, _result_box=None):
    global _NC_CACHE
    if _NC_CACHE is None:
        _NC_CACHE = build_nc()
    nc = _NC_CACHE

    q = np.ascontiguousarray(q, dtype=np.float32)
    y = np.ascontiguousarray(y, dtype=np.float32)
    experts = np.ascontiguousarray(experts, dtype=np.float32)
    gate_w = np.ascontiguousarray(gate_w, dtype=np.float32)
    gate_b = np.ascontiguousarray(gate_b, dtype=np.float32)

    in_maps = []
    for i in range(NCORES):
        sl = slice(i * BPC, (i + 1) * BPC)
        in_maps.append({
            "q": q[sl], "y": y[sl],
            "experts": experts, "gate_w": gate_w, "gate_b": gate_b,
        })

    kwargs = {}
    if _trace:
        kwargs = dict(trace=True, trace_cores=[0])
    res = run_bass_kernel_spmd(nc, in_maps, core_ids=list(range(NCORES)), **kwargs)
    if _result_box is not None:
        _result_box.append(res)
    return np.concatenate([res.results[i]["out"] for i in range(NCORES)], axis=0)


# revision 6
# speedup vs baseline: 1.1143x; 1.1143x over previous
"""MoE-routed per-sample conv2d kernel for Trainium2 (8 NeuronCores, SPMD).

Math (per sample b):
    y_ctx  = mean(y[b], HW)                              [C]
    gates  = softmax(y_ctx @ (gate_w[:C] + gate_w[C:]) + gate_b)   [E]
    Wf[e]  = experts[e,:, :C] + experts[e,:, C:]         [O, C, K, K]  (fold of q;q concat)
    agg    = sum_e gates[e] * Wf[e]
    out[b] = conv2d(q[b], agg, SAME)

Sharding: data-parallel over batch. Each of the 8 cores handles B/8 = 2
samples; experts/gate params replicated. Conv runs on the TensorEngine as
9 shifted matmuls (one per kernel tap) accumulated in PSUM, fp32r.

Engine/ring roles:
  SP (sync)      HWDGE load ring A: e0, y chunks, a third of the q chunks
  ACT (scalar)   HWDGE ring B: e1 + half of y0 at startup, then ONLY the
                 output writes; engine does wft drains, y accumulates
                 (interleaved by predicted arrival), exp, PSUM->SBUF copies
  Pool (gpsimd)  SWDGE ring C: tiny loads + first q chunks + half of y1;
                 kept off the startup-critical y0 (SWDGE is slower)
  DVE (vector)   y reduces (SP's half), gating vector ops, agg FMAs, err subs
  PE (tensor)    folded expert transposes (accumulating), gating matmuls, conv
"""

import numpy as np

import concourse.bass as bass
import concourse.tile as tile
from concourse import bacc, mybir
from concourse.bass_utils import run_bass_kernel_spmd
from concourse.masks import make_identity
from concourse.tile_rust import add_dep_helper

F32 = mybir.dt.float32
F32R = mybir.dt.float32r

B, C, O, H, W, E, K = 16, 128, 128, 128, 128, 3, 3
NCORES = 8
BPC = B // NCORES          # samples per core
CH_ROWS = 32               # output rows per conv chunk
NCH = H // CH_ROWS         # chunks per sample (4)
RB_ROWS = 4                # output rows per PSUM block (4*128 = 512 free)
NRB = CH_ROWS // RB_ROWS   # row blocks per chunk (8)
HCH = CH_ROWS // 2         # rows per output write (half chunk)
XCF = 2 + (CH_ROWS + 3) * W      # flat chunk tile: 2 lead zeros, 35 rows, slack
YCHUNK = 1024              # y columns per reduce chunk (0.5 MB)
NYCH = (H * W) // YCHUNK   # 16

# taps ordered so the first one covers the full output range (ky=1,kx=1)
TAPS = [(1, 1)] + [(ky, kx) for ky in range(3) for kx in range(3) if (ky, kx) != (1, 1)]
# agg tap-groups sized [2,3,4]: a small first group lets the conv start early
AGG_GROUPS = [slice(0, 2), slice(2, 5), slice(5, 9)]

MUL = mybir.AluOpType.mult
ADD = mybir.AluOpType.add


def build_nc():
    nc = bacc.Bacc(None, target_bir_lowering=False)

    q_d = nc.dram_tensor("q", [BPC, C, H, W], F32, kind="ExternalInput")
    y_d = nc.dram_tensor("y", [BPC, C, H, W], F32, kind="ExternalInput")
    ex_d = nc.dram_tensor("experts", [E, O, 2 * C, K, K], F32, kind="ExternalInput")
    gw_d = nc.dram_tensor("gate_w", [2 * C, E], F32, kind="ExternalInput")
    gb_d = nc.dram_tensor("gate_b", [E], F32, kind="ExternalInput")
    out_d = nc.dram_tensor("out", [BPC, O, H, W], F32, kind="ExternalOutput")

    with tile.TileContext(nc) as tc:
        import contextlib

        with contextlib.ExitStack() as ctx:
            const = ctx.enter_context(tc.tile_pool(name="const", bufs=1))
            wraw = ctx.enter_context(tc.tile_pool(name="wraw", bufs=3))
            wft = ctx.enter_context(tc.tile_pool(name="wft", bufs=3))
            ypool = ctx.enter_context(tc.tile_pool(name="ypool", bufs=6))
            gp = ctx.enter_context(tc.tile_pool(name="gp", bufs=4))
            atmp = ctx.enter_context(tc.tile_pool(name="atmp", bufs=1))
            aggp = ctx.enter_context(tc.tile_pool(name="aggp", bufs=2))
            xcp = ctx.enter_context(tc.tile_pool(name="xcp", bufs=4))
            osbp = ctx.enter_context(tc.tile_pool(name="osbp", bufs=4))
            psp = ctx.enter_context(tc.tile_pool(name="psp", bufs=6, space="PSUM"))
            pse = ctx.enter_context(tc.tile_pool(name="pse", bufs=2, space="PSUM"))

            # Keep each DMA ring's transfer order exactly as emitted: the
            # static Tile scheduler otherwise floats "ready" loads ahead of
            # y chunks, starving the gating path.
            last_dma = {}

            def chained_dma(eng, out, in_):
                inst = eng.dma_start(out=out, in_=in_)
                key = eng.engine
                if key in last_dma:
                    add_dep_helper(inst.ins, last_dma[key], sync=False,
                                   reason="ring FIFO order")
                last_dma[key] = inst.ins
                return inst

            # ---- tiny loads + constants (SWDGE ring) -----------------------
            gw = const.tile([C, 2, E], F32, tag="gw", name="gw")
            chained_dma(nc.gpsimd, gw[:], gw_d[:].rearrange("(h c) e -> c h e", h=2))
            gbt = const.tile([1, E], F32, tag="gbt", name="gbt")
            chained_dma(nc.gpsimd, gbt[:], gb_d[:].rearrange("(x e) -> x e", x=1))

            # expert loads: e0 on SP, e1 on ACT, e2 on SWDGE
            wes = []
            for e, eng in ((0, nc.sync), (1, nc.scalar), (2, nc.gpsimd)):
                we = wraw.tile([O, 2 * C, K, K], F32, tag="wraw", name=f"we{e}")
                chained_dma(eng, we[:], ex_d[e])
                wes.append(we)

            # ---- y0 loads: 8 chunks on SP + 8 on ACT (both HWDGE) ----------
            yflat = y_d[:].rearrange("b c h w -> b c (h w)")
            y0_cs = []
            for j in range(NYCH):
                yc = ypool.tile([C, YCHUNK], F32, tag="yc", name=f"yc0_{j}")
                chained_dma(nc.sync if j < 8 else nc.scalar, yc[:],
                            yflat[0, :, j * YCHUNK:(j + 1) * YCHUNK])
                y0_cs.append(yc)

            ident = const.tile([128, 128], F32, tag="ident", name="ident")
            make_identity(nc, ident)

            # prewarm the ACT Exp table so gating doesn't pay the table load
            warm = const.tile([1, 1], F32, tag="warm", name="warm")
            nc.vector.memset(warm[:], 0.0)
            nc.scalar.activation(warm[:], warm[:], mybir.ActivationFunctionType.Exp,
                                 bias=0.0, scale=1.0)

            ones = const.tile([1, 128], F32, tag="ones", name="ones")
            nc.vector.memset(ones[:], 1.0)

            weff = const.tile([C, E], F32, tag="weff", name="weff")
            nc.vector.tensor_add(weff[:], gw[:, 0, :], gw[:, 1, :])
            # fold the 1/HW of the y-mean into the gate weight
            nc.vector.tensor_scalar_mul(weff[:], weff[:], 1.0 / float(H * W))

            # ---- y partial sums --------------------------------------------
            yparts = []

            def new_ypart():
                ypart = gp.tile([C, NYCH], F32, tag="ypart",
                                name=f"ypart{len(yparts)}")
                yparts.append(ypart)

            new_ypart()

            def reduce_dve(b, ycs, js):
                for j in js:
                    nc.vector.reduce_sum(yparts[b][:, j:j + 1], ycs[j][:],
                                         axis=mybir.AxisListType.X)

            def accum_act(b, ycs, js):
                # ACT-side reduce: activation-copy with running accumulator
                for j in js:
                    nc.scalar.activation(
                        ycs[j][:], ycs[j][:], mybir.ActivationFunctionType.Copy,
                        accum_out=yparts[b][:, j:j + 1])

            # ---- expert transpose with in-PE fold --------------------------
            # agg lhsT layout [c, t, o]; fold of the duplicated input halves
            # done by two accumulating PE transposes per tap.  ACT drains are
            # interleaved with the y0 accumulates by predicted arrival order
            # so neither starves the other.
            reduce_dve(0, y0_cs, range(0, 8))
            wfts = []
            act_accum_plan = {0: [8, 9], 1: [10, 11, 12], 2: [13, 14, 15]}
            for e in range(E):
                we = wes[e]
                wt = wft.tile([C, K * K, O], F32, tag="wft", name=f"wft{e}")
                for t, (ky, kx) in enumerate(TAPS):
                    pst = psp.tile([128, 128], F32, tag="ps", name=f"pst{e}_{t}")
                    nc.tensor.matmul(pst[:], we[:, 0:C, ky, kx], ident[:],
                                     is_transpose=True, start=True, stop=False)
                    nc.tensor.matmul(pst[:], we[:, C:2 * C, ky, kx], ident[:],
                                     is_transpose=True, start=False, stop=True)
                    nc.scalar.copy(wt[:, t, :], pst[:])
                wfts.append(wt)
                accum_act(0, y0_cs, act_accum_plan[e])

            # ---- q chunk staging -------------------------------------------
            xcs = {}

            def load_xc(b, ch, eng):
                xr_lo = max(0, CH_ROWS * ch - 1)
                xr_hi = min(H - 1, CH_ROWS * ch + CH_ROWS)
                nrows = xr_hi - xr_lo + 1
                j0 = xr_lo - (CH_ROWS * ch - 1)
                xc = xcp.tile([C, XCF], F32R, tag="xc", name=f"xc{b}_{ch}")
                nc.gpsimd.memset(xc[:, 0:2].bitcast(F32), 0.0)
                nc.gpsimd.memset(
                    xc[:, 2 + (CH_ROWS + 2) * W: 2 + (CH_ROWS + 2) * W + 2].bitcast(F32), 0.0)
                if ch == 0:
                    nc.gpsimd.memset(xc[:, 2:2 + W].bitcast(F32), 0.0)
                if ch == NCH - 1:
                    nc.gpsimd.memset(
                        xc[:, 2 + (CH_ROWS + 1) * W: 2 + (CH_ROWS + 2) * W].bitcast(F32), 0.0)
                chained_dma(
                    eng,
                    xc[:, 2 + j0 * W: 2 + (j0 + nrows) * W],
                    q_d[b, :, xr_lo:xr_hi + 1, :].rearrange("c h w -> c (h w)").bitcast(F32R),
                )
                xcs[(b, ch)] = xc

            load_xc(0, 0, nc.gpsimd)
            load_xc(0, 1, nc.gpsimd)

            # ---- gating + weight aggregation per sample --------------------
            aggs = []

            def gate_and_agg(b):
                ysum = gp.tile([C, 1], F32, tag="ysum", name=f"ysum{b}")
                nc.vector.reduce_sum(ysum[:], yparts[b][:],
                                     axis=mybir.AxisListType.X)
                ps13 = pse.tile([1, E], F32, tag="pse", name=f"ps13_{b}")
                nc.tensor.matmul(ps13[:], ysum[:], weff[:], start=True, stop=True)
                logits = gp.tile([1, E], F32, tag="logits", name=f"logits{b}")
                nc.vector.tensor_add(logits[:], ps13[:], gbt[:])
                mx = gp.tile([1, 1], F32, tag="mx", name=f"mx{b}")
                nc.vector.reduce_max(mx[:], logits[:], axis=mybir.AxisListType.X)
                nc.vector.tensor_scalar_mul(mx[:], mx[:], -1.0)
                nc.scalar.activation(logits[:], logits[:], mybir.ActivationFunctionType.Exp,
                                     bias=mx[:], scale=1.0)
                sm = gp.tile([1, 1], F32, tag="sm", name=f"sm{b}")
                nc.vector.reduce_sum(sm[:], logits[:], axis=mybir.AxisListType.X)
                nc.vector.reciprocal(sm[:], sm[:])
                nc.vector.tensor_scalar_mul(logits[:], logits[:], sm[:])
                # broadcast gates to all partitions via a K=1 matmul with ones
                psg = pse.tile([128, E], F32, tag="pse", name=f"psg{b}")
                nc.tensor.matmul(psg[:], ones[:], logits[:], start=True, stop=True)
                gbc = gp.tile([128, E], F32, tag="gbc", name=f"gbc{b}")
                nc.vector.tensor_copy(gbc[:], psg[:])

                # aggregate in tap-groups; a mul plus two DVE FMAs per group
                accf = atmp.tile([C, K * K, O], F32, tag="accf", name=f"accf{b}")
                agg = aggp.tile([C, K * K, O], F32R, tag="agg", name=f"agg{b}")
                for sl in AGG_GROUPS:
                    nc.vector.tensor_scalar_mul(accf[:, sl, :], wfts[0][:, sl, :],
                                                gbc[:, 0:1])
                    nc.vector.scalar_tensor_tensor(
                        accf[:, sl, :], wfts[1][:, sl, :], gbc[:, 1:2],
                        accf[:, sl, :], MUL, ADD)
                    nc.vector.scalar_tensor_tensor(
                        agg[:, sl, :], wfts[2][:, sl, :], gbc[:, 2:3],
                        accf[:, sl, :], MUL, ADD)
                aggs.append(agg)

            # ---- conv ------------------------------------------------------
            # Main taps read the flat chunk at offset 2 + (4rb+ky)*W + kx-1.
            # For kx=0 the first column of each row wrongly reads the last
            # element of the previous row (and vice versa for kx=2), which
            # SAME-padding says should be zero.  err matmuls compute exactly
            # those wrong contributions; they are subtracted on the SBUF copy.
            def conv_chunk(b, ch, err_late=False):
                last = (b == BPC - 1) and (ch == NCH - 1)
                xc = xcs[(b, ch)]
                x1 = xc[:, 1:1 + (CH_ROWS + 2) * W].rearrange("c (r w) -> c r w", w=W)
                x2 = xc[:, 2:2 + (CH_ROWS + 3) * W].rearrange("c (r w) -> c r w", w=W)
                # err psum [O, 2, CH_ROWS]: group 0 = col 0, group 1 = col W-1
                errps = pse.tile([O, 2, CH_ROWS], F32, tag="pse", name=f"eps{b}_{ch}")

                def emit_errs():
                    first = True
                    for t, (ky, kx) in enumerate(TAPS):
                        if kx == 1:
                            continue
                        if kx == 0:
                            g, rhs = 0, x1[:, ky:ky + CH_ROWS, 0:1]
                        else:
                            g, rhs = 1, x2[:, ky + 1:ky + 1 + CH_ROWS, 0:1]
                        nc.tensor.matmul(
                            errps[:, g, :], aggs[b][:, t, :], rhs,
                            start=first, stop=(t == len(TAPS) - 1),
                            skip_group_check=True,
                        )
                        first = False

                if not err_late:
                    emit_errs()
                # two half-chunk output stages, each its own SBUF tile + write
                halves = []
                for hh in range(2):
                    osb = osbp.tile([O, HCH, W], F32, tag="osb",
                                    name=f"osb{b}_{ch}_{hh}")
                    for rb in range(hh * NRB // 2, (hh + 1) * NRB // 2):
                        ps = psp.tile([O, RB_ROWS, W], F32, tag="ps",
                                      name=f"ps{b}_{ch}_{rb}")
                        for t, (ky, kx) in enumerate(TAPS):
                            jb = RB_ROWS * rb + ky
                            off = 2 + jb * W + kx - 1
                            rhs = xc[:, off:off + RB_ROWS * W]  # contiguous 512
                            nc.tensor.matmul(
                                ps[:],
                                aggs[b][:, t, :],
                                rhs,
                                start=(t == 0),
                                stop=(t == len(TAPS) - 1),
                            )
                        osl = slice(RB_ROWS * rb - hh * HCH,
                                    RB_ROWS * (rb + 1) - hh * HCH)
                        esl = slice(RB_ROWS * rb, RB_ROWS * (rb + 1))
                        nc.scalar.copy(osb[:, osl, :], ps[:])
                        if last and hh == 1:
                            # drain the tail per row-block to cut the drain
                            nc.vector.tensor_sub(osb[:, osl, 0], osb[:, osl, 0],
                                                 errps[:, 0, esl])
                            nc.vector.tensor_sub(osb[:, osl, W - 1],
                                                 osb[:, osl, W - 1],
                                                 errps[:, 1, esl])
                            r0 = CH_ROWS * ch + RB_ROWS * rb
                            chained_dma(nc.scalar,
                                        out_d[b, :, r0:r0 + RB_ROWS, :],
                                        osb[:, osl, :])
                    halves.append((hh, osb))
                if err_late:
                    emit_errs()
                for hh, osb in halves:
                    if last and hh == 1:
                        continue
                    esl = slice(hh * HCH, (hh + 1) * HCH)
                    nc.vector.tensor_sub(osb[:, :, 0], osb[:, :, 0],
                                         errps[:, 0, esl])
                    nc.vector.tensor_sub(osb[:, :, W - 1], osb[:, :, W - 1],
                                         errps[:, 1, esl])
                    r0 = CH_ROWS * ch + hh * HCH
                    chained_dma(nc.scalar, out_d[b, :, r0:r0 + HCH, :], osb[:])

            # ---- schedule --------------------------------------------------
            gate_and_agg(0)
            # y1 loads ride SP/SWDGE while sample-0 convs run
            y1_cs = []
            new_ypart()
            for j in range(NYCH):
                yc = ypool.tile([C, YCHUNK], F32, tag="yc", name=f"yc1_{j}")
                chained_dma(nc.sync if j < 8 else nc.gpsimd, yc[:],
                            yflat[1, :, j * YCHUNK:(j + 1) * YCHUNK])
                y1_cs.append(yc)
            load_xc(0, 2, nc.sync)
            load_xc(0, 3, nc.gpsimd)
            conv_chunk(0, 0, err_late=True)
            load_xc(1, 0, nc.sync)
            load_xc(1, 1, nc.gpsimd)
            conv_chunk(0, 1)
            reduce_dve(1, y1_cs, range(NYCH))
            load_xc(1, 2, nc.sync)
            load_xc(1, 3, nc.gpsimd)
            conv_chunk(0, 2)
            gate_and_agg(1)
            conv_chunk(0, 3)
            for ch in range(NCH):
                conv_chunk(1, ch)

    nc.compile()
    return nc


_NC_CACHE = None


def kernel(q, y, experts, gate_w, gate_b, _trace=False, _result_box=None):
    global _NC_CACHE
    if _NC_CACHE is None:
        _NC_CACHE = build_nc()
    nc = _NC_CACHE

    q = np.ascontiguousarray(q, dtype=np.float32)
    y = np.ascontiguousarray(y, dtype=np.float32)
    experts = np.ascontiguousarray(experts, dtype=np.float32)
    gate_w = np.ascontiguousarray(gate_w, dtype=np.float32)
    gate_b = np.ascontiguousarray(gate_b, dtype=np.float32)

    in_maps = []
    for i in range(NCORES):
        sl = slice(i * BPC, (i + 1) * BPC)
        in_maps.append({
            "q": q[sl], "y": y[sl],
            "experts": experts, "gate_w": gate_w, "gate_b": gate_b,
        })

    kwargs = {}
    if _trace:
        kwargs = dict(trace=True, trace_cores=[0])
    res = run_bass_kernel_spmd(nc, in_maps, core_ids=list(range(NCORES)), **kwargs)
    if _result_box is not None:
        _result_box.append(res)
    return np.concatenate([res.results[i]["out"] for i in range(NCORES)], axis=0)
